# revision 47
# baseline (speedup 1.0000x reference)
"""Handshaking kernel ('cat' type) for Trainium2, 8 NeuronCores.

Math: for each upper-triangular pair (i, j>=i):
    out[b, p(i,j), :] = tanh(W1 @ h_i + W2 @ h_j + bias),  W = [W1 | W2]

Decomposition: per-token projections A = seq @ W1^T + bias and C = seq @ W2^T
(small fp16 matmuls), then out[p(i,j)] = tanh(A[i] + C[j]).

Key layout fact: pair rows are slab-contiguous -- slab i (= pairs (i, i..L-1))
occupies output rows [OFF[i], OFF[i+1]).  The whole output is the ordered
concatenation of slabs, so we produce it as 257 full [128, HH] tiles of
CONSECUTIVE pair rows and write them with plain contiguous DMAs (no scatter,
no gpsimd, no per-row DVE work).  Every PSUM tile is built purely on the PE
from full [K=128 -> M=128] matmuls (no partial-partition ops, so no PE
quadrant alignment issues):
  - the leading run's C rows via shifted-identity "band" matmuls (slices of
    one constant [128, 384] fp16 shifted identity; out-of-range rows clip
    to zero, which is exact for leading runs),
  - everything else via host-precomputed 0/1 selector matrices (fp8e4m3,
    exact): Sc gathers C[j(m)] rows, Sa gathers A[i(m)] rows, one matmul
    per (matrix, source half), accumulated in PSUM,
  - tiles whose C side would need >=2 matmuls instead use ONE fp8 DoubleRow
    pair selector [S0|S1] against packed [C0|C1] sources split hi/mid/lo in
    fp8 (3 half-rate matmuls = 1.5 normal matmuls for the whole C side, to
    ~1e-3 accuracy),
  - tiles fully inside i,j >= 192 use a single selector over a packed
    source MIX = [C1[64:]; A1[64:]] built once in the preamble.
Selectors stream from DRAM in 1 MB column chunks, loaded just-in-time
against a per-group column watermark and overlapped with compute;
an explicit PE "fence" matmul per chunk (reading the chunk as the tracked
moving operand) guarantees the PE never reads a chunk before its DMA lands
(the framework does not reliably sync weights-operand reads).  PSUM is two
4-bank [128, 4, 512] tiles ping-ponged, ACT does tanh over 4 banks per
instruction while evicting to fp16 SBUF staging, and one DMA per 8 tiles
streams staging to DRAM.  Output is fp16 (abs err ~2e-3 << 2e-2 tol); the
host upcasts to fp32.  A dozen rank-1 warmup matmuls raise the PE p-state
while inputs load.

Sharding: 8 cores = 4 batches x 2 halves of the hidden dim (H=768 -> 384 per
core). All cores run the identical program (SPMD).
"""

import sys
import numpy as np

for _p in ("/opt/trn_rl_repo", "/root/.axon_site/_ro/trn_rl_repo"):
    if _p not in sys.path:
        sys.path.insert(0, _p)

SELT_FP8 = True      # selectors as fp8e4m3 (0/1 exact) to halve their DMA

B, L, H = 4, 256, 768
HH = H // 2          # per-core hidden slice
NPAIR = L * (L + 1) // 2   # 32896
NT = NPAIR // 128    # 257 full output tiles
GRP = 8              # tiles per staging buffer / output DMA

# offset of pair (i, i) in the flattened pair dim; pair (i, j) -> OFF[i] + j - i
OFF = np.array([i * L - (i * (i - 1)) // 2 for i in range(L + 1)], dtype=np.int64)

_r = np.arange(NPAIR)
I_OF = (np.searchsorted(OFF, _r, side="right") - 1).astype(np.int64)
J_OF = (_r - OFF[I_OF] + I_OF).astype(np.int64)


def build_schedule():
    """Per-tile matmul plans, identical for every core.

    Tile t covers pair rows [128t, 128t+128).  Each tile is a dict with:
      bands: list of (e, h) -- psum[m] += C_h[e+m] for e+m in [0,128)
             (lhsT = padib[:, 128+e : 256+e], rhs = C half h)
      sels:  list of ("c"|"a", h, S) -- psum += S.T @ (C|A half h),
             S a [128,128] 0/1 matrix (stored in the big selector tensor)
    The leading run of each tile is rendered with bands when exact under
    clipping; all other content goes through selector matrices.
    """
    tiles = []
    for t in range(NT):
        rows = slice(128 * t, 128 * (t + 1))
        ii, jj = I_OF[rows], J_OF[rows]
        cut = np.flatnonzero(np.diff(ii)) + 1
        starts = np.concatenate([[0], cut])
        ends = np.concatenate([cut, [128]])
        pieces = [(int(s), int(e)) for s, e in zip(starts, ends)]

        m = np.arange(128)
        # --- A side: selector per touched half ---
        sels = []
        for h in (0, 1):
            msk = (ii // 128) == h
            if msk.any():
                S = np.zeros((128, 128), np.float32)
                S[ii[msk] - 128 * h, m[msk]] = 1.0
                sels.append(("a", h, S))

        # --- C side ---
        # leading piece [0, ln): try bands (split at the C-half boundary);
        # each sub-band is exact iff clipping zeroes everything outside it.
        s0, e0 = pieces[0]
        lead = np.zeros(128, bool)
        lead[s0:e0] = True
        bands = []
        band_rows = np.zeros(128, bool)
        for h in (0, 1):
            msk = lead & ((jj // 128) == h)
            if not msk.any():
                continue
            mm = m[msk]
            e = int(jj[mm[0]] - 128 * h - mm[0])
            # clip-exactness: e+m in [0,128) exactly on msk
            ok = ((e + m >= 0) & (e + m < 128)) == msk
            if ok.all():
                bands.append((e, h))
                band_rows |= msk
        # everything not band-covered -> C selectors per half; if a half
        # needs a selector anyway, fold that half's band into it (saves a MM)
        rest = ~band_rows
        for h in (0, 1):
            msk = rest & ((jj // 128) == h)
            if msk.any():
                folded = [b for b in bands if b[1] == h]
                if folded:
                    bands = [b for b in bands if b[1] != h]
                    msk = (jj // 128) == h
                S = np.zeros((128, 128), np.float32)
                S[jj[msk] - 128 * h, m[msk]] = 1.0
                sels.append(("c", h, S))

        if ii[0] >= 192:
            # all i,j in [192,256): one selector over MIX = [C1[64:]; A1[64:]]
            S = np.zeros((128, 128), np.float32)
            S[jj - 192, m] = 1.0
            S[ii - 192 + 64, m] += 1.0
            bands, sels = [], [("x", 0, S)]
        else:
            ncs = sum(k == "c" for k, _h, _S in sels)
            cmms = len(bands) + ncs
            # C side with >=2 matmuls can become one fp8 DoubleRow pair
            # selector (S0 over C0 | S1 over C1): hi/mid/lo = 3 half-rate
            # matmuls (1.5 normal-matmul cost for the WHOLE C side).  Skip
            # where the extra selector bytes cost more DMA than the PE win
            # (2-band tiles; every 4th band+sel tile) -- DMA device balance.
            if cmms >= 2:
                Sp = np.zeros((128, 256), np.float32)
                Sp[jj[jj < 128], m[jj < 128]] = 1.0
                Sp[jj[jj >= 128] - 128, 128 + m[jj >= 128]] = 1.0
                bands = []
                sels = [s for s in sels if s[0] != "c"] + [("p", 0, Sp)]
        tiles.append(dict(bands=bands, sels=sels, npieces=len(pieces)))
    return tiles


TILES = build_schedule()
# assign column offsets in the big selector tensor ("p" entries are 256 wide)
_col = 0
for _t in TILES:
    _t["soff"] = []
    for _k, _h, _S in _t["sels"]:
        _t["soff"].append(_col)
        _col += _S.shape[1]
SELCOLS = _col
NSEL = sum(len(t["sels"]) for t in TILES)
NMM = sum(len(t["sels"]) + len(t["bands"]) for t in TILES)


def _schedule_selfcheck():
    """Verify the schedule reproduces A[i] + C[j] exactly (incl. clipping)."""
    rng = np.random.RandomState(0)
    Cc = rng.randn(L, 8)
    Aa = rng.randn(L, 8)
    Mx = np.concatenate([Cc[192:], Aa[192:]], axis=0)
    CA = {"c": Cc, "a": Aa, "x": Mx}
    got = np.full((NPAIR, 8), np.nan)
    m = np.arange(128)
    for t, tl in enumerate(TILES):
        ps = np.zeros((128, 8))
        for e, h in tl["bands"]:
            src = e + m
            ok = (src >= 0) & (src < 128)
            ps[ok] += Cc[128 * h + src[ok]]
        for kind, h, S in tl["sels"]:
            if kind == "p":
                ps += S[:, :128].T @ Cc[:128] + S[:, 128:].T @ Cc[128:]
            else:
                src_ = Mx if kind == "x" else CA[kind][128 * h:128 * (h + 1)]
                ps += S.T @ src_
        got[128 * t:128 * (t + 1)] = ps
    ii, jj = np.triu_indices(L)
    assert np.allclose(got, Aa[ii] + Cc[jj]), "schedule self-check failed"


_schedule_selfcheck()

_CACHE = {}


def _build_nc():
    import concourse.bass as bass
    import concourse.bacc as bacc
    import concourse.mybir as mybir
    import concourse.tile as tile

    f32 = mybir.dt.float32
    f16 = mybir.dt.float16
    seldt = mybir.dt.float8e4 if SELT_FP8 else f16

    nc = bacc.Bacc(None, target_bir_lowering=False, debug=False)

    seqT = nc.dram_tensor("seqT", [H, L], f16, kind="ExternalInput")
    w1t = nc.dram_tensor("w1t", [H, HH], f16, kind="ExternalInput")
    w2t = nc.dram_tensor("w2t", [H, HH], f16, kind="ExternalInput")
    bias = nc.dram_tensor("bias", [1, HH], f16, kind="ExternalInput")
    ones = nc.dram_tensor("ones", [1, 128], f16, kind="ExternalInput")
    padib = nc.dram_tensor("padib", [128, 384], f16, kind="ExternalInput")
    selt = nc.dram_tensor("selt", [128, SELCOLS], seldt, kind="ExternalInput")
    out = nc.dram_tensor("out", [NPAIR, HH], f16, kind="ExternalOutput")

    SELCH = 8192   # selector COLUMNS per load chunk (overlaps load w/ compute)

    with tile.TileContext(nc) as tc:
        with (
            tc.tile_pool(name="persist", bufs=1) as pers,
            tc.tile_pool(name="outp", bufs=6) as outp,
        ):
            seqT_sb = pers.tile([128, 6 * L], f16, tag="seqT", name="seqT")
            w1t_sb = pers.tile([128, 6 * HH], f16, tag="w1t", name="w1t")
            w2t_sb = pers.tile([128, 6 * HH], f16, tag="w2t", name="w2t")
            bias_sb = pers.tile([1, HH], f16, tag="bias")
            ones_sb = pers.tile([1, 128], f16, tag="ones")
            padib_sb = pers.tile([128, 384], f16, tag="padib")
            selt_sb = pers.tile([128, SELCOLS], seldt, tag="selt")

            nc.sync.dma_start(ones_sb[:], ones[:])
            nc.sync.dma_start(padib_sb[:], padib[:])
            nc.sync.dma_start(
                seqT_sb[:], bass.AP(seqT, 0, [[L, 128], [128 * L, 6], [1, L]])
            )
            nc.sync.dma_start(
                w2t_sb[:], bass.AP(w2t, 0, [[HH, 128], [128 * HH, 6], [1, HH]])
            )
            nc.sync.dma_start(
                w1t_sb[:], bass.AP(w1t, 0, [[HH, 128], [128 * HH, 6], [1, HH]])
            )
            nc.sync.dma_start(bias_sb[:], bias[:])

            nchunk = -(-SELCOLS // SELCH)

            def load_sel_chunk(k):
                if k < nchunk:
                    c0, c1 = k * SELCH, min(SELCOLS, (k + 1) * SELCH)
                    nc.sync.dma_start(selt_sb[:, c0:c1], selt[:, c0:c1])

            load_sel_chunk(0)
            load_sel_chunk(1)
            loaded = 2

            # ---- PE p-state warmup: dummy rank-1 matmuls while inputs load ----
            wu_ctx = tc.tile_pool(name="wu_ps", bufs=1, space="PSUM")
            wu_ps = wu_ctx.__enter__()
            wups = wu_ps.tile([128, 128], f32, tag="wu")
            for _ in range(12):
                nc.tensor.matmul(
                    wups[:], lhsT=ones_sb[:1, :], rhs=ones_sb[:1, :],
                    start=True, stop=True,
                )
            # fences: PE waits the seqT load before any weights read
            nc.tensor.matmul(
                wups[:], lhsT=ones_sb[:1, :], rhs=seqT_sb[:1, :128],
                start=True, stop=True,
            )
            wu_ctx.__exit__(None, None, None)

            # ---- precompute C = seq @ W2^T, A = seq @ W1^T + bias (-> f16) ----
            pre_ctx = tc.tile_pool(name="pre_ps", bufs=4, space="PSUM")
            pre_ps = pre_ctx.__enter__()
            srcs = {}
            c_psum = {}
            for name, wt, add_b, toff in (
                ("c0", w2t_sb, False, 0),
                ("c1", w2t_sb, False, 128),
                ("a0", w1t_sb, True, 0),
                ("a1", w1t_sb, True, 128),
            ):
                ps = pre_ps.tile([128, HH], f32, tag="pre")
                for k in range(6):
                    nc.tensor.matmul(
                        ps[:],
                        lhsT=seqT_sb[:, k * L + toff:k * L + toff + 128],
                        rhs=wt[:, k * HH:(k + 1) * HH],
                        start=(k == 0),
                        stop=(k == 5 and not add_b),
                    )
                if add_b:
                    nc.tensor.matmul(
                        ps[:], lhsT=ones_sb[:1, :], rhs=bias_sb[:1, :],
                        start=False, stop=True,
                    )
                dst = pers.tile([128, HH], f16, tag=name, name=name)
                nc.vector.tensor_copy(dst[:], ps[:])
                srcs[name] = dst
                if name in ("c0", "c1"):
                    c_psum[int(name[1])] = ps

            # fp8 hi/mid/lo split of [C0|C1] pairs for DoubleRow matmuls
            cph = pers.tile([128, 2, HH], seldt, tag="cph", name="cph")
            cpm = pers.tile([128, 2, HH], seldt, tag="cpm", name="cpm")
            cpl = pers.tile([128, 2, HH], seldt, tag="cpl", name="cpl")
            tmp1 = pers.tile([128, HH], f32, tag="tmp1")
            tmp2 = pers.tile([128, HH], f32, tag="tmp2")
            for h in (0, 1):
                cps = c_psum[h]
                nc.vector.tensor_copy(cph[:, h, :], cps[:])
                nc.vector.tensor_sub(tmp1[:], cps[:], cph[:, h, :])
                nc.vector.tensor_copy(cpm[:, h, :], tmp1[:])
                nc.vector.tensor_sub(tmp2[:], tmp1[:], cpm[:, h, :])
                nc.vector.tensor_copy(cpl[:, h, :], tmp2[:])
            cp_terms = (cph, cpm, cpl)

            # MIX source: rows 0..63 = C1[64:] (band, clipped), rows 64..127
            # = A1[64:] (partition-aligned DVE copy over the zero rows)
            psx = pre_ps.tile([128, HH], f32, tag="pre")
            # fence: PE waits the padib load before its first weights read
            nc.tensor.matmul(
                psx[:, :128], lhsT=padib_sb[:1, :128], rhs=padib_sb[:1, :128],
                start=True, stop=True,
            )
            nc.tensor.matmul(
                psx[:], lhsT=padib_sb[:, 192:320], rhs=srcs["c1"][:],
                start=True, stop=True,
            )
            dstx = pers.tile([128, HH], f16, tag="x0", name="x0")
            nc.vector.tensor_copy(dstx[:, :], psx[:, :])
            nc.vector.tensor_copy(dstx[64:128, :], srcs["a1"][64:128, :])
            srcs["x0"] = dstx

            pre_ctx.__exit__(None, None, None)
            mm_ctx = tc.tile_pool(name="mm_ps", bufs=2, space="PSUM")
            mm_ps = mm_ctx.__enter__()

            # ---- main loop: 257 output tiles in GRP-sized staging groups,
            # PSUM as [128, 4, 512] 4-bank tiles, ACT batched over 4 banks ----
            sel_at = 0
            fenced = 0   # chunks [0, fenced) are PE-fenced
            bounds = list(range(0, NT - 9, GRP)) + [NT - 9, NT - 5, NT - 3, NT - 1, NT]
            # per-group high-water column, for just-in-time chunk loads
            gend = []
            for g in range(len(bounds) - 1):
                cols = [tl["soff"][-1] + tl["sels"][-1][2].shape[1]
                        for tl in TILES[bounds[g]:bounds[g + 1]] if tl["sels"]]
                gend.append(max(cols) if cols else (gend[-1] if gend else 0))
            for g in range(1, len(gend)):
                gend[g] = max(gend[g], gend[g - 1])
            for g, g0 in enumerate(bounds[:-1]):
                need = gend[min(g + 3, len(gend) - 1)]
                while loaded * SELCH < need and loaded < nchunk:
                    load_sel_chunk(loaded)
                    loaded += 1
                ng = bounds[g + 1] - g0
                ot = outp.tile([128, GRP, HH], f16, tag="ot")
                for b0 in range(0, ng, 4):
                    nb = min(4, ng - b0)
                    ps4 = mm_ps.tile([128, 4, 512], f32, tag="mm")
                    # fence any selector chunk this 4-tile batch will touch:
                    # a throwaway matmul whose ifmap (tracked) reads the chunk
                    # makes the PE queue wait for the chunk's DMA.
                    last_col = max(
                        (TILES[g0 + b0 + b]["soff"][-1] +
                         TILES[g0 + b0 + b]["sels"][-1][2].shape[1]
                         for b in range(nb)
                         if TILES[g0 + b0 + b]["sels"]),
                        default=0,
                    )
                    while fenced * SELCH < last_col and fenced < nchunk:
                        co = fenced * SELCH
                        nc.tensor.matmul(
                            ps4[:, 0:1, :128],
                            lhsT=selt_sb[:, co:co + 128],
                            rhs=selt_sb[:, co:co + 128],
                            start=True, stop=True,
                        )
                        fenced += 1
                    for bi in range(nb):
                        tl = TILES[g0 + b0 + bi]
                        psb = ps4[:, bi:bi + 1, :HH]
                        nmm = len(tl["bands"]) + sum(
                            3 if k == "p" else 1 for k, _h, _S in tl["sels"]
                        )
                        mi = 0
                        for e, h in tl["bands"]:
                            nc.tensor.matmul(
                                psb, lhsT=padib_sb[:, 128 + e:256 + e],
                                rhs=srcs[f"c{h}"][:],
                                start=(mi == 0), stop=(mi == nmm - 1),
                            )
                            mi += 1
                        for (kind, h, _S), co in zip(tl["sels"], tl["soff"]):
                            sel_at += 1
                            if kind == "p":
                                lw = selt_sb[:, co:co + 256].rearrange(
                                    "p (two m) -> p two m", two=2
                                )
                                for ti, term in enumerate(cp_terms):
                                    nc.tensor.matmul(
                                        psb, lhsT=lw, rhs=term[:, :, :],
                                        perf_mode=mybir.MatmulPerfMode.DoubleRow,
                                        start=(mi == 0), stop=(mi == nmm - 1),
                                    )
                                    mi += 1
                            else:
                                nc.tensor.matmul(
                                    psb, lhsT=selt_sb[:, co:co + 128],
                                    rhs=srcs[f"{kind}{h}"][:],
                                    start=(mi == 0), stop=(mi == nmm - 1),
                                )
                                mi += 1
                    nc.scalar.activation(
                        ot[:, b0:b0 + nb, :], ps4[:, :nb, :HH],
                        mybir.ActivationFunctionType.Tanh,
                    )
                # one contiguous DMA for the whole group:
                # DRAM row 128*(g0+gi) + p  <->  staging[p, gi, :]
                dst = bass.AP(out, 128 * g0 * HH,
                              [[HH, 128], [128 * HH, ng], [1, HH]])
                src = ot[:, :ng, :]
                nc.sync.dma_start(dst, src)
            assert sel_at == NSEL

            mm_ctx.__exit__(None, None, None)

    nc.compile()
    return nc


def _get_nc():
    if "nc" not in _CACHE:
        _CACHE["nc"] = _build_nc()
    return _CACHE["nc"]


def _host_consts():
    if "consts" in _CACHE:
        return _CACHE["consts"]
    import ml_dtypes
    padi = np.zeros((128, 384), np.float32)
    for k in range(128):
        padi[k, k + 128] = 1.0
    selt = np.empty((128, SELCOLS), np.float32)
    for tl in TILES:
        for (_kind, _h, S), co in zip(tl["sels"], tl["soff"]):
            selt[:, co:co + S.shape[1]] = S
    consts = dict(
        padib=padi.astype(np.float16),
        selt=selt.astype(ml_dtypes.float8_e4m3 if SELT_FP8 else np.float16),
        ones=np.ones((1, 128), np.float16),
    )
    _CACHE["consts"] = consts
    return consts


def _core_inputs(seq_hiddens, W, b):
    """Per-core input maps (core c = batch c//2, hidden half c%2)."""
    consts = _host_consts()
    w1T = np.ascontiguousarray(W[:, :H].T)   # [H(k), H(h)]
    w2T = np.ascontiguousarray(W[:, H:].T)

    in_maps = []
    for c in range(8):
        bb, hf = divmod(c, 2)
        hs = slice(hf * HH, (hf + 1) * HH)
        in_maps.append(
            {
                "seqT": np.ascontiguousarray(seq_hiddens[bb].T).astype(np.float16),
                "w1t": np.ascontiguousarray(w1T[:, hs]).astype(np.float16),
                "w2t": np.ascontiguousarray(w2T[:, hs]).astype(np.float16),
                "bias": np.ascontiguousarray(b[hs])[None, :].astype(np.float16),
                **consts,
            }
        )
    return in_maps


def kernel(seq_hiddens, W, b):
    from concourse.bass_utils import run_bass_kernel_spmd

    seq_hiddens = np.asarray(seq_hiddens, dtype=np.float32)
    W = np.asarray(W, dtype=np.float32)
    b = np.asarray(b, dtype=np.float32)

    nc = _get_nc()
    in_maps = _core_inputs(seq_hiddens, W, b)

    res = run_bass_kernel_spmd(nc, in_maps, list(range(8)))
    full = np.empty((B, NPAIR, H), np.float32)
    for bb in range(B):
        full[bb, :, :HH] = np.asarray(res.results[2 * bb]["out"], dtype=np.float32)
        full[bb, :, HH:] = np.asarray(res.results[2 * bb + 1]["out"], dtype=np.float32)
    return full


if __name__ == "__main__":
    rng = np.random.RandomState(0)
    sh = rng.randn(B, L, H).astype(np.float32)
    Wv = (rng.randn(H, 2 * H) * 0.02).astype(np.float32)
    bv = np.zeros(H, np.float32)
    o = kernel(seq_hiddens=sh, W=Wv, b=bv)
    print("kernel output", o.shape, o.dtype, float(np.abs(o).max()))
